# revision 6
# baseline (speedup 1.0000x reference)
"""Multi-head attention (B=2, N=2048, C=1024, H=16) on 8 Trainium2 cores.

Sharding: core cid = (b, hg) with b = cid//4, hg = cid%4.  Data-parallel on
batch, 4-way tensor-parallel on heads (4 heads / 256 dims per core).  Each
core computes q/k/v projections for its head slice, full (masked-softmax)
attention for its 4 heads, and a partial output projection y^T = Wp_slice^T
-contracted over its 256 dims.  Host sums the 4 partials per batch and adds
the proj bias.

v2 pipeline (vs the phase-serial v1):
  - Everything bf16 on-device (f32 PSUM accumulation); host pre-casts x/W.
  - QKV phase-A chunks and attention blocks interleave: attention block
    (hp=0, ncb=0) starts right after phase-A chunk 0; per-(tile, chunk)
    SBUF tiles keep the dependency tracking fine-grained.
  - Scores S^T = K^T_h x Q^T_h per head-pair packed at partition offsets
    0/64 (K=64 each); exp on Scalar; mask multiply split DVE (par0) /
    GpSimd (par1); PV lags 4 m-tiles behind scores.
  - PSUM: pool2 (2-bank) x3 shared by phase-A segments + score ping-pong;
    pool1 (1-bank) x2 for O^T accumulators / proj-C outputs.
  - Softmax denominator via ones-column in V_aug; normalize with
    reciprocal + partition broadcast.
"""

import os
import sys
import types
from contextlib import ExitStack

import numpy as np
import ml_dtypes

import concourse.bass as bass
import concourse.mybir as mybir
import concourse.tile as tile
from concourse import bacc
from concourse.bass_utils import run_bass_kernel_spmd

# ---------------------------------------------------------------- constants
N = 2048          # sequence length
C = 1024          # model dim
NH = 4            # heads per core
HD = 64           # head dim
DQK = 2 * NH * HD # 512: q rows then k rows in qk^T
DV = NH * HD      # 256
NCK = 512         # n-chunk size
NCH = N // NCK    # 4 n-chunks
MT = N // 128     # 16 m-tiles
CK = C // 128     # 8 contraction chunks
SCALE = HD ** -0.5
NCORES = 8
LAG = 4           # PV trails scores by this many m-tiles

F32 = mybir.dt.float32
BF16 = mybir.dt.bfloat16


def _ensure_ntff_hook():
    """bass_utils' trace path imports antenv.axon_hooks, which this image
    lacks; inject it and register the ctypes-based NTFF profile hook."""
    if "antenv.axon_hooks" in sys.modules:
        return
    mod = types.ModuleType("antenv.axon_hooks")
    _hook = [None]
    mod.set_axon_ntff_profile_hook = lambda h: _hook.__setitem__(0, h)
    mod.get_axon_ntff_profile_hook = lambda: _hook[0]
    sys.modules["antenv.axon_hooks"] = mod
    try:
        from trn_agent_boot.trn_boot import _ntff_profile_via_ctypes

        mod.set_axon_ntff_profile_hook(
            _ntff_profile_via_ctypes("/opt/axon/libaxon_pjrt.so")
        )
    except Exception:
        pass


def build():
    nc = bacc.Bacc("TRN2", target_bir_lowering=False, debug=False,
                   num_devices=NCORES)
    xT = nc.dram_tensor("xT", [C, N], BF16, kind="ExternalInput")
    wqk = nc.dram_tensor("wqkT", [C, DQK], BF16, kind="ExternalInput")
    wv = nc.dram_tensor("wvT", [C, DV], BF16, kind="ExternalInput")
    wp = nc.dram_tensor("wpT", [DV, C], BF16, kind="ExternalInput")
    mk = nc.dram_tensor("maskT", [N, N], mybir.dt.uint8, kind="ExternalInput")
    yT = nc.dram_tensor("yT", [C, N], F32, kind="ExternalOutput")

    with tile.TileContext(nc) as tc, ExitStack() as ctx:
        consts = ctx.enter_context(tc.tile_pool(name="consts", bufs=1))
        xin = ctx.enter_context(tc.tile_pool(name="xin", bufs=16))
        m8p = ctx.enter_context(tc.tile_pool(name="m8p", bufs=4))
        ptp = ctx.enter_context(tc.tile_pool(name="ptp", bufs=6))
        ysb = ctx.enter_context(tc.tile_pool(name="ysb", bufs=3))
        dnp = ctx.enter_context(tc.tile_pool(name="dnp", bufs=2))
        rbp = ctx.enter_context(tc.tile_pool(name="rbp", bufs=2))
        # PSUM: 3x 2-bank + 2x 1-bank tiles = 8 banks static.
        pool2 = ctx.enter_context(tc.tile_pool(name="pool2", bufs=3, space="PSUM"))
        pool1 = ctx.enter_context(tc.tile_pool(name="pool1", bufs=2, space="PSUM"))

        # ---- resident weights (wp loaded late, after phase-A x traffic)
        wqk_sb = consts.tile([128, CK, DQK], BF16)
        wv_sb = consts.tile([128, CK, DV], BF16)
        wp_sb = consts.tile([128, 2, C], BF16)
        nc.sync.dma_start(out=wqk_sb,
                          in_=wqk[:].rearrange("(co ci) d -> ci co d", ci=128))
        nc.sync.dma_start(out=wv_sb,
                          in_=wv[:].rearrange("(co ci) d -> ci co d", ci=128))

        # ---- per-chunk / per-mtile intermediates (fine-grained deps)
        # qkT[m][c]: rows m*128..m*128+128 of qk^T, columns c*512..(c+1)*512
        qkT = [[consts.tile([128, NCK], BF16, name=f"qk_m{m}_c{c}")
                for c in range(NCH)] for m in range(4)]
        vb = [consts.tile([128, NH, 128], BF16, name=f"vb_{mt}")
              for mt in range(MT)]
        mask_sb = [consts.tile([128, N], BF16, name=f"mask_m{mt}")
                   for mt in range(MT)]
        ot_sb = [consts.tile([128, 2, NCK], BF16, name=f"ot_n{ncb}")
                 for ncb in range(NCH)]

        # V_aug ones column + zero pad (during DMA head; engines idle)
        for mt in range(MT):
            eng = nc.vector if mt % 2 == 0 else nc.gpsimd
            eng.memset(vb[mt][:, :, HD:], 0.0)
            eng.memset(vb[mt][:, :, HD:HD + 1], 1.0)

        # PE p-state warmup under the initial DMAs
        warm = consts.tile([128, NCK], BF16, name="warm")
        nc.vector.memset(warm[:, 0:NCK], 0.0)
        pwarm = pool1.tile([128, NCK], F32, tag="p1", name="pwarm")
        for i in range(14):
            nc.tensor.matmul(pwarm, lhsT=warm[:, 0:128], rhs=warm,
                             start=True, stop=True)

        # mask m-tiles 0..3 early: attention block (0,0) needs them first
        def emit_mask(mts):
            for mt in mts:
                m8 = m8p.tile([128, N], mybir.dt.uint8)
                nc.sync.dma_start(out=m8, in_=mk[mt * 128:(mt + 1) * 128, :])
                nc.vector.tensor_copy(out=mask_sb[mt], in_=m8)

        # ---------------- phase A chunk: q/k then v, 2-bank PSUM segments
        def emit_A(c):
            xts = []
            for cc in range(CK):
                xt = xin.tile([128, NCK], BF16)
                nc.sync.dma_start(
                    out=xt[:, 0:NCK // 2],
                    in_=xT[cc * 128:(cc + 1) * 128,
                           c * NCK:c * NCK + NCK // 2])
                nc.sync.dma_start(
                    out=xt[:, NCK // 2:],
                    in_=xT[cc * 128:(cc + 1) * 128,
                           c * NCK + NCK // 2:(c + 1) * NCK])
                xts.append(xt)
            for g in range(2):
                pa = pool2.tile([128, 2, NCK], F32, tag="p2")
                for cc in range(CK):
                    for mm in range(2):
                        m = 2 * g + mm
                        nc.tensor.matmul(
                            pa[:, mm, :],
                            lhsT=wqk_sb[:, cc, m * 128:(m + 1) * 128],
                            rhs=xts[cc], start=(cc == 0), stop=(cc == CK - 1))
                for mm in range(2):
                    m = 2 * g + mm
                    if mm == 0:
                        nc.scalar.copy(out=qkT[m][c], in_=pa[:, mm, :])
                    else:
                        nc.vector.tensor_copy(out=qkT[m][c], in_=pa[:, mm, :])
            for seg in range(2):
                pv = pool2.tile([128, 2, NCK], F32, tag="p2")
                for cc in range(CK):
                    for jj in range(2):
                        j = 2 * seg + jj
                        nc.tensor.matmul(
                            pv[:, jj, 0:DV],
                            lhsT=xts[cc][:, j * 128:(j + 1) * 128],
                            rhs=wv_sb[:, cc, :],
                            start=(cc == 0), stop=(cc == CK - 1))
                for jj in range(2):
                    mt = 4 * c + 2 * seg + jj
                    src = pv[:, jj, 0:DV].rearrange("p (h d) -> p h d", h=NH)
                    if jj == 0:
                        nc.scalar.copy(out=vb[mt][:, :, 0:HD], in_=src)
                    else:
                        nc.vector.tensor_copy(out=vb[mt][:, :, 0:HD], in_=src)

        # ---------------- attention block (hp, ncb): 16 m-tile sweep
        class Block:
            def __init__(self, hp, ncb):
                self.hp, self.ncb = hp, ncb
                self.nsl = slice(ncb * NCK, (ncb + 1) * NCK)
                self.pso = None
                self.pts = {}

            def _pv(self, mt):
                pt = self.pts.pop(mt)
                for par in range(2):
                    nc.tensor.matmul(
                        self.pso[par],
                        lhsT=vb[mt][:, 2 * self.hp + par, :],
                        rhs=pt[:, par, :],
                        start=(mt == 0), stop=(mt == MT - 1))

            def steps(self, mts):
                if self.pso is None:
                    self.pso = [pool1.tile([128, NCK], F32, tag="p1",
                                           name=f"pso{self.hp}_{self.ncb}_{p}")
                                for p in range(2)]
                mq, mkt = self.hp, 2 + self.hp
                for mt in mts:
                    pss = pool2.tile([128, 2, NCK], F32, tag="p2")
                    for par in range(2):
                        po = par * 64
                        nc.tensor.matmul(
                            pss[:, par, :],
                            lhsT=qkT[mkt][mt // 4][po:po + 64,
                                                   (mt % 4) * 128:
                                                   (mt % 4 + 1) * 128],
                            rhs=qkT[mq][self.ncb][po:po + 64, :],
                            start=True, stop=True)
                    pt = ptp.tile([128, 2, NCK], BF16)
                    nc.scalar.activation(
                        out=pt, in_=pss,
                        func=mybir.ActivationFunctionType.Exp, scale=SCALE)
                    for par in range(2):
                        nc.vector.tensor_mul(out=pt[:, par, :],
                                             in0=pt[:, par, :],
                                             in1=mask_sb[mt][:, self.nsl])
                    self.pts[mt] = pt
                    if mt >= LAG:
                        self._pv(mt - LAG)

            def finalize(self):
                for mt in sorted(self.pts):
                    self._pv(mt)
                for par in range(2):
                    po = par * 64
                    den = dnp.tile([1, NCK], F32, tag="den")
                    nc.vector.tensor_copy(out=den,
                                          in_=self.pso[par][HD:HD + 1, :])
                    rec = dnp.tile([1, NCK], F32, tag="rec")
                    nc.vector.reciprocal_approx_fast(out=rec, in_=den)
                    rb = rbp.tile([64, NCK], F32)
                    nc.gpsimd.partition_broadcast(rb, rec)
                    nc.vector.tensor_mul(
                        out=ot_sb[self.ncb][po:po + 64, self.hp, :],
                        in0=self.pso[par][0:HD, :], in1=rb)

        # ---------------- output projection partial for one n-chunk
        def emit_proj(ncb):
            nsl = slice(ncb * NCK, (ncb + 1) * NCK)
            for et in range(8):
                psy = pool1.tile([128, NCK], F32, tag="p1")
                for dk in range(2):
                    nc.tensor.matmul(
                        psy,
                        lhsT=wp_sb[:, dk, et * 128:(et + 1) * 128],
                        rhs=ot_sb[ncb][:, dk, :],
                        start=(dk == 0), stop=(dk == 1))
                yt = ysb.tile([128, NCK], F32)
                nc.vector.tensor_copy(out=yt, in_=psy)
                nc.sync.dma_start(out=yT[et * 128:(et + 1) * 128, nsl],
                                  in_=yt)

        # ---------------- schedule
        b0 = Block(0, 0)
        emit_mask([0, 1])
        emit_A(0)
        emit_mask([2, 3, 4, 5])
        b0.steps(range(0, 4))
        emit_A(1)
        emit_mask([6, 7, 8, 9])
        b0.steps(range(4, 8))
        emit_A(2)
        emit_mask([10, 11, 12, 13])
        b0.steps(range(8, 12))
        emit_A(3)
        emit_mask([14, 15])
        nc.sync.dma_start(out=wp_sb,
                          in_=wp[:].rearrange("(dk ci) e -> ci dk e", ci=128))
        b0.steps(range(12, 16))
        b0.finalize()
        for ncb in range(NCH):
            for hp in range(2):
                if (hp, ncb) == (0, 0):
                    continue
                blk = Block(hp, ncb)
                blk.steps(range(MT))
                blk.finalize()
            emit_proj(ncb)

    nc.compile()
    return nc


_NC = None


def _get_nc():
    global _NC
    if _NC is None:
        _NC = build()
    return _NC


def make_in_maps(x, mask, W_qkv, W_proj):
    x = np.asarray(x, dtype=np.float32)
    mask = np.asarray(mask)
    W_qkv = np.asarray(W_qkv, dtype=np.float32)
    W_proj = np.asarray(W_proj, dtype=np.float32)
    bf = ml_dtypes.bfloat16
    in_maps = []
    for cid in range(NCORES):
        b, hg = divmod(cid, 4)
        rs = slice(hg * 256, (hg + 1) * 256)
        wq = W_qkv[0 * C:1 * C][rs]          # [256, 1024]
        wk = W_qkv[1 * C:2 * C][rs]
        wvs = W_qkv[2 * C:3 * C][rs]
        in_maps.append({
            "xT": np.ascontiguousarray(x[b].T).astype(bf),
            "wqkT": np.ascontiguousarray(
                np.concatenate([wq, wk], axis=0).T).astype(bf),
            "wvT": np.ascontiguousarray(wvs.T).astype(bf),
            "wpT": np.ascontiguousarray(W_proj[:, rs].T).astype(bf),
            "maskT": np.ascontiguousarray(mask[b, 0].T).astype(np.uint8),
        })
    return in_maps


LAST_EXEC_NS = None
LAST_MEAN_EXEC_NS = None


def kernel(x, mask, W_qkv, W_proj, b_proj):
    global LAST_EXEC_NS, LAST_MEAN_EXEC_NS
    trace = bool(int(os.environ.get("TRNK_TRACE", "0")))
    if trace:
        _ensure_ntff_hook()
    nc = _get_nc()
    in_maps = make_in_maps(x, mask, W_qkv, W_proj)
    res = run_bass_kernel_spmd(nc, in_maps, list(range(NCORES)), trace=trace)
    LAST_EXEC_NS = res.exec_time_ns
    LAST_MEAN_EXEC_NS = res.mean_exec_time_ns
    y = np.zeros((2, N, C), dtype=np.float32)
    for cid in range(NCORES):
        b = cid // 4
        y[b] += np.asarray(res.results[cid]["yT"], dtype=np.float32).T
    y += np.asarray(b_proj, dtype=np.float32)[None, None, :]
    return y
